# revision 18
# baseline (speedup 1.0000x reference)
"""BERT self-attention (B=4, S=2048, D=1024, H=16) on 8 trn2 NeuronCores.

Sharding: core c -> (batch b = c//2, head-group hg = c%2, 8 heads each).
Each core computes out[b, :, hg*512:(hg+1)*512]; host gathers. Inputs are
pre-transposed AND cast to bf16 on host so matmuls run at full PE rate
(fp32 feed is half rate): xt = X.T [D,S] bf16, w{q,k,v}t = W.T [D,512] bf16.

On-device per core, all matmul operands bf16, PSUM accumulation fp32:
  Q^T, K^T: [o, s] pair-tiles (2 heads / 128 partitions); V_aug [s, h, 65]
  with a leading ones column per head (row 0 of ctx psum = softmax denom).
  Scores S^T[j, i] per head pair packed into one [128, 2, 512] PSUM tile
  (the two heads' K=64 matmuls run concurrently in disjoint PE row groups).
  U = exp(0.125*S^T + mask[j]) in ONE activation instr over 1024 free
  elems -> bf16 SBUF.
  ctx_aug[1+dh, i] accumulated over all 16 j-tiles directly in PSUM.
  Drain: DVE reciprocal of denom row, GpSimd partition-broadcast, DVE
  multiply -> bf16 out tile. Output is stored TRANSPOSED [o, s] in DRAM;
  the host transposes for free during the gather.
  Q/K projections for pair p+1 are interleaved into pair p's attention
  loop (1 matmul per score-tile slot) to fill the ACT-bound PE slack.
"""

import numpy as np
import ml_dtypes

import concourse.bass as bass
import concourse.tile as tile
from concourse import bacc, mybir
from concourse.bass_utils import run_bass_kernel_spmd

B, S, D, H = 4, 2048, 1024, 16
DH = 64
O = 512  # per-core output width (8 heads)
HL = 8  # local heads per core
NP = 4  # head pairs per core
ST = S // 128  # 16 s-tiles
NIC = 4  # i-chunks of 512
F32 = mybir.dt.float32
BF16 = mybir.dt.bfloat16
EXP = mybir.ActivationFunctionType.Exp

_NC_CACHE = None


def build_nc():
    nc = bacc.Bacc(
        "TRN2",
        target_bir_lowering=False,
        debug=False,
        enable_asserts=True,
        num_devices=8,
    )
    xt = nc.dram_tensor("xt", [D, S], BF16, kind="ExternalInput").ap()
    wqt = nc.dram_tensor("wqt", [D, O], BF16, kind="ExternalInput").ap()
    wkt = nc.dram_tensor("wkt", [D, O], BF16, kind="ExternalInput").ap()
    wvt = nc.dram_tensor("wvt", [D, O], BF16, kind="ExternalInput").ap()
    bq = nc.dram_tensor("bq", [O], F32, kind="ExternalInput").ap()
    bk = nc.dram_tensor("bk", [O], F32, kind="ExternalInput").ap()
    bv = nc.dram_tensor("bv", [O], F32, kind="ExternalInput").ap()
    mask = nc.dram_tensor("mask", [S], F32, kind="ExternalInput").ap()
    # transposed output: [o, s]; host transposes during gather
    out = nc.dram_tensor("out", [O, S], BF16, kind="ExternalOutput").ap()

    with tile.TileContext(nc) as tc:
        _emit(nc, tc, xt, wqt, wkt, wvt, bq, bk, bv, mask, out)
    nc.compile()
    return nc


def _emit(nc, tc, xt, wqt, wkt, wvt, bq, bk, bv, mask, out):
    with (
        tc.tile_pool(name="singles", bufs=1) as singles,
        tc.tile_pool(name="persist", bufs=1) as persist,
        tc.tile_pool(name="work", bufs=1) as work,
        tc.tile_pool(name="psum", bufs=1, space="PSUM") as psum,
    ):
        mask_sb = singles.tile([128, ST], F32)
        nc.sync.dma_start(out=mask_sb, in_=mask.rearrange("(t p) -> p t", p=128))
        bq_sb = singles.tile([128, NP], F32)
        nc.sync.dma_start(out=bq_sb, in_=bq.rearrange("(t p) -> p t", p=128))
        bk_sb = singles.tile([128, NP], F32)
        nc.sync.dma_start(out=bk_sb, in_=bk.rearrange("(t p) -> p t", p=128))
        bv_bc = singles.tile([128, O], F32)
        nc.sync.dma_start(
            out=bv_bc, in_=bass.AP(tensor=bv.tensor, offset=0, ap=[[0, 128], [1, O]])
        )
        # warm the exp table set while the input DMAs stream
        warm_in = singles.tile([128, 1], F32)
        nc.vector.memset(warm_in, 0.0)
        warm_out = singles.tile([128, 1], F32)
        nc.scalar.activation(warm_out, warm_in, EXP)

        # persistent activations. vaug layout per head: col 0 = ones (softmax
        # denominator via the ctx matmul), cols 1:64 = zero pad (PE partition
        # slices must start 32-aligned, so V sits at rows 64:128 of ctx psum),
        # cols 64:128 = V + bias.
        qts = [persist.tile([128, S], BF16, name=f"qt{p}", tag=f"qt{p}") for p in range(NP)]
        kts = [persist.tile([128, S], BF16, name=f"kt{p}", tag=f"kt{p}") for p in range(NP)]
        vaug = [
            persist.tile([128, HL, 128], BF16, name=f"vaug{t}", tag=f"vaug{t}")
            for t in range(ST)
        ]

        # input tiles (kept resident; wk/x first so K/Q proj of pair 0 starts early)
        def load_w(wdram, label):
            wts = []
            for dt in range(8):
                w = work.tile([128, O], BF16, name=f"w{label}{dt}", tag=f"w{label}{dt}")
                nc.sync.dma_start(out=w, in_=wdram[dt * 128 : (dt + 1) * 128, :])
                wts.append(w)
            return wts

        # DMA order: wk, first i-slice of x (k c=0 can start), wq (q c=0),
        # wv (V projection feeds pair-0 chunk 0), then the rest of x
        wk_t = load_w(wkt, "k")
        xts = [work.tile([128, S], BF16, name=f"xts{dt}", tag=f"xts{dt}") for dt in range(8)]
        for dt in range(8):
            nc.sync.dma_start(out=xts[dt][:, 0:512], in_=xt[dt * 128 : (dt + 1) * 128, 0:512])
        wq_t = load_w(wqt, "q")
        wv_t = load_w(wvt, "v")
        for dt in range(8):
            nc.sync.dma_start(out=xts[dt][:, 512:S], in_=xt[dt * 128 : (dt + 1) * 128, 512:S])

        def kq_proj_steps(p, c0=0, kq_outer=False):
            """Generator: Q/K projection of pair p, one matmul per yield.
            kq_outer emits all K chunks before all Q chunks (K columns are
            read by every j-tile of the pair's first chunk, Q per i-chunk)."""
            kq = ((wk_t, kts, bk_sb), (wq_t, qts, bq_sb))
            cs = range(c0, 4)
            order = (
                [(w, c) for w in kq for c in cs]
                if kq_outer
                else [(w, c) for c in cs for w in kq]
            )
            for (wts, dsts, bias_sb), c in order:
                ps = psum.tile([128, 512], F32, name=f"pp{p}_{c}", tag="pp", bufs=2)
                for dt in range(8):
                    nc.tensor.matmul(
                        ps,
                        wts[dt][:, p * 128 : (p + 1) * 128],
                        xts[dt][:, c * 512 : (c + 1) * 512],
                        start=(dt == 0),
                        stop=(dt == 7),
                    )
                    if dt == 7:
                        nc.vector.tensor_scalar_add(
                            dsts[p][:, c * 512 : (c + 1) * 512],
                            ps,
                            bias_sb[:, p : p + 1],
                        )
                    yield

        def vproj_steps():
            """Generator: V projection + V_aug assembly, one s-tile group per
            yield. vaug[st][:, h, 0] = 1, [:, h, 64:128] = V + bv."""
            for st in range(ST):
                ps = psum.tile([128, O], F32, name=f"ppv{st}", tag="pp", bufs=2)
                for dt in range(8):
                    nc.tensor.matmul(
                        ps,
                        xts[dt][:, st * 128 : (st + 1) * 128],
                        wv_t[dt],
                        start=(dt == 0),
                        stop=(dt == 7),
                    )
                va = vaug[st]
                nc.vector.memset(va[:, :, 0:1], 1.0)
                nc.vector.memset(va[:, :, 1:DH], 0.0)
                nc.vector.tensor_add(
                    va[:, :, DH : 2 * DH],
                    ps.rearrange("p (h d) -> p h d", h=HL),
                    bv_bc.rearrange("p (h d) -> p h d", h=HL),
                )
                yield True

        # only pair-0 c=0 (16 matmuls) up front; everything else is a single
        # filler chain consumed 1-2 matmuls per attention slot
        import itertools

        for _ in itertools.islice(kq_proj_steps(0), 16):
            pass
        vgen = vproj_steps()
        chain = itertools.chain(
            kq_proj_steps(0, c0=1, kq_outer=True),
            kq_proj_steps(1),
            kq_proj_steps(2),
            kq_proj_steps(3),
        )

        # attention per pair, with projections interleaved
        for p in range(NP):
            for ic in range(NIC):
                isl = slice(ic * 512, (ic + 1) * 512)
                cxs = [
                    psum.tile([128, 512], F32, name=f"cx{p}_{ic}_{x}", tag="cx", bufs=2)
                    for x in range(2)
                ]
                def emit_ctx(jt, u):
                    for x in range(2):
                        nc.tensor.matmul(
                            cxs[x],
                            vaug[jt][:, 2 * p + x, :],
                            u[:, x, :],
                            start=(jt == 0),
                            stop=(jt == ST - 1),
                        )

                prev = None
                for jt in range(ST):
                    # projection filler first: these chain writes must be
                    # EMITTED before any scores that read them (emission order
                    # defines program order for the dependency tracker)
                    next(chain, None)
                    next(chain, None)
                    # V projection rides in pair 0's first two chunks, one
                    # s-tile group per slot, ahead of the ctx that needs it
                    next(vgen, None)
                    s = psum.tile(
                        [128, 2, 512], F32, name=f"s{p}_{ic}_{jt}", tag="s", bufs=2
                    )
                    for x in range(2):
                        hp = slice(x * 64, (x + 1) * 64)
                        nc.tensor.matmul(
                            s[:, x, :],
                            kts[p][hp, jt * 128 : (jt + 1) * 128],
                            qts[p][hp, isl],
                            start=True,
                            stop=True,
                            tile_position=(x * 64, 0),
                        )
                    u = work.tile([128, 2, 512], BF16, name=f"u{p}_{ic}_{jt}", tag="u", bufs=12)
                    nc.scalar.activation(
                        u.rearrange("p x i -> p (x i)"),
                        s.rearrange("p x i -> p (x i)"),
                        EXP,
                        bias=mask_sb[:, jt : jt + 1],
                        scale=0.125,
                    )
                    # ctx one slot behind: its exp has already finished, so the
                    # PE queue never blocks a full exp latency mid-slot
                    if prev is not None:
                        emit_ctx(*prev)
                    prev = (jt, u)
                emit_ctx(*prev)
                # drain: row 0 of cxs = softmax denominator, rows 64:128 = ctx.
                # Copy PSUM->SBUF immediately (releases the bank in ~0.4us so
                # the next chunk's ctx accumulation isn't stalled), then
                # normalize off the critical path.
                for x in range(2):
                    st_ = work.tile([128, 512], F32, name=f"st{p}_{ic}_{x}", tag="st", bufs=3)
                    nc.vector.tensor_copy(out=st_, in_=cxs[x])
                    rd = work.tile([1, 512], F32, name=f"rd{p}_{ic}_{x}", tag="rd", bufs=2)
                    nc.vector.reciprocal_approx_fast(out=rd, in_=st_[0:1, :])
                    rdb = work.tile([128, 512], F32, name=f"rdb{p}_{ic}_{x}", tag="rdb", bufs=2)
                    nc.gpsimd.partition_broadcast(rdb, rd, channels=128)
                    ob = work.tile([128, 512], BF16, name=f"ob{p}_{ic}_{x}", tag="ob", bufs=3)
                    nc.vector.tensor_mul(
                        ob[DH:128, :], st_[DH:128, :], rdb[DH:128, :]
                    )
                    hh = 2 * p + x
                    nc.sync.dma_start(
                        out=out[hh * DH : (hh + 1) * DH, isl], in_=ob[DH:128, :]
                    )


def _make_in_maps(hidden_states, attention_mask, Wq, bq, Wk, bk, Wv, bv):
    bf = ml_dtypes.bfloat16
    in_maps = []
    for c in range(8):
        b, hg = divmod(c, 2)
        sl = slice(hg * O, (hg + 1) * O)
        in_maps.append(
            {
                "xt": np.ascontiguousarray(hidden_states[b].T).astype(bf),
                "wqt": np.ascontiguousarray(Wq[sl, :].T).astype(bf),
                "wkt": np.ascontiguousarray(Wk[sl, :].T).astype(bf),
                "wvt": np.ascontiguousarray(Wv[sl, :].T).astype(bf),
                "bq": np.ascontiguousarray(bq[sl]),
                "bk": np.ascontiguousarray(bk[sl]),
                "bv": np.ascontiguousarray(bv[sl]),
                "mask": np.ascontiguousarray(attention_mask[b, 0, 0, :]),
            }
        )
    return in_maps


def _gather(results):
    out = np.empty((B, S, D), dtype=np.float32)
    for c in range(8):
        b, hg = divmod(c, 2)
        out[b, :, hg * O : (hg + 1) * O] = results[c]["out"].astype(np.float32).T
    return out


def kernel(hidden_states, attention_mask, Wq, bq, Wk, bk, Wv, bv, **run_kwargs):
    global _NC_CACHE
    args = [hidden_states, attention_mask, Wq, bq, Wk, bk, Wv, bv]
    args = [np.asarray(a, dtype=np.float32) for a in args]
    if _NC_CACHE is None:
        _NC_CACHE = build_nc()
    in_maps = _make_in_maps(*args)
    res = run_bass_kernel_spmd(_NC_CACHE, in_maps, core_ids=list(range(8)), **run_kwargs)
    kernel.last_result = res
    return _gather(res.results)


# revision 21
# speedup vs baseline: 1.0220x; 1.0220x over previous
"""BERT self-attention (B=4, S=2048, D=1024, H=16) on 8 trn2 NeuronCores.

Sharding: core c -> (batch b = c//2, head-group hg = c%2, 8 heads each).
Each core computes out[b, :, hg*512:(hg+1)*512]; host gathers. Inputs are
pre-transposed AND cast to bf16 on host so matmuls run at full PE rate
(fp32 feed is half rate): xt = X.T [D,S] bf16, w{q,k,v}t = W.T [D,512] bf16.

On-device per core, all matmul operands bf16, PSUM accumulation fp32:
  Q^T, K^T: [o, s] pair-tiles (2 heads / 128 partitions); V_aug [s, h, 65]
  with a leading ones column per head (row 0 of ctx psum = softmax denom).
  Scores S^T[j, i] per head pair packed into one [128, 2, 512] PSUM tile
  (the two heads' K=64 matmuls run concurrently in disjoint PE row groups).
  U = exp(0.125*S^T + mask[j]) in ONE activation instr over 1024 free
  elems -> bf16 SBUF.
  ctx_aug[1+dh, i] accumulated over all 16 j-tiles directly in PSUM.
  Drain: DVE reciprocal of denom row, GpSimd partition-broadcast, DVE
  multiply -> bf16 out tile. Output is stored TRANSPOSED [o, s] in DRAM;
  the host transposes for free during the gather.
  Q/K projections for pair p+1 are interleaved into pair p's attention
  loop (1 matmul per score-tile slot) to fill the ACT-bound PE slack.
"""

import numpy as np
import ml_dtypes

import concourse.bass as bass
import concourse.tile as tile
from concourse import bacc, mybir
from concourse.bass_utils import run_bass_kernel_spmd

B, S, D, H = 4, 2048, 1024, 16
DH = 64
O = 512  # per-core output width (8 heads)
HL = 8  # local heads per core
NP = 4  # head pairs per core
ST = S // 128  # 16 s-tiles
NIC = 4  # i-chunks of 512
F32 = mybir.dt.float32
BF16 = mybir.dt.bfloat16
EXP = mybir.ActivationFunctionType.Exp

_NC_CACHE = None


def build_nc():
    nc = bacc.Bacc(
        "TRN2",
        target_bir_lowering=False,
        debug=False,
        enable_asserts=True,
        num_devices=8,
    )
    xt = nc.dram_tensor("xt", [D, S], BF16, kind="ExternalInput").ap()
    wqt = nc.dram_tensor("wqt", [D, O], BF16, kind="ExternalInput").ap()
    wkt = nc.dram_tensor("wkt", [D, O], BF16, kind="ExternalInput").ap()
    wvt = nc.dram_tensor("wvt", [D, O], BF16, kind="ExternalInput").ap()
    bq = nc.dram_tensor("bq", [O], F32, kind="ExternalInput").ap()
    bk = nc.dram_tensor("bk", [O], F32, kind="ExternalInput").ap()
    bv = nc.dram_tensor("bv", [O], F32, kind="ExternalInput").ap()
    mask = nc.dram_tensor("mask", [S], F32, kind="ExternalInput").ap()
    # transposed output: [o, s]; host transposes during gather
    out = nc.dram_tensor("out", [O, S], BF16, kind="ExternalOutput").ap()

    with tile.TileContext(nc) as tc:
        _emit(nc, tc, xt, wqt, wkt, wvt, bq, bk, bv, mask, out)
    nc.compile()
    return nc


def _emit(nc, tc, xt, wqt, wkt, wvt, bq, bk, bv, mask, out):
    with (
        tc.tile_pool(name="singles", bufs=1) as singles,
        tc.tile_pool(name="persist", bufs=1) as persist,
        tc.tile_pool(name="work", bufs=1) as work,
        tc.tile_pool(name="psum", bufs=1, space="PSUM") as psum,
    ):
        mask_sb = singles.tile([128, ST], F32)
        nc.sync.dma_start(out=mask_sb, in_=mask.rearrange("(t p) -> p t", p=128))
        bq_sb = singles.tile([128, NP], F32)
        nc.sync.dma_start(out=bq_sb, in_=bq.rearrange("(t p) -> p t", p=128))
        bk_sb = singles.tile([128, NP], F32)
        nc.sync.dma_start(out=bk_sb, in_=bk.rearrange("(t p) -> p t", p=128))
        bv_bc = singles.tile([128, O], F32)
        nc.sync.dma_start(
            out=bv_bc, in_=bass.AP(tensor=bv.tensor, offset=0, ap=[[0, 128], [1, O]])
        )
        # warm the exp table set while the input DMAs stream
        warm_in = singles.tile([128, 1], F32)
        nc.vector.memset(warm_in, 0.0)
        warm_out = singles.tile([128, 1], F32)
        nc.scalar.activation(warm_out, warm_in, EXP)

        # persistent activations. vaug layout per head: col 0 = ones (softmax
        # denominator via the ctx matmul), cols 1:64 = zero pad (PE partition
        # slices must start 32-aligned, so V sits at rows 64:128 of ctx psum),
        # cols 64:128 = V + bias.
        qts = [persist.tile([128, S], BF16, name=f"qt{p}", tag=f"qt{p}") for p in range(NP)]
        kts = [persist.tile([128, S], BF16, name=f"kt{p}", tag=f"kt{p}") for p in range(NP)]
        vaug = [
            persist.tile([128, HL, 128], BF16, name=f"vaug{t}", tag=f"vaug{t}")
            for t in range(ST)
        ]

        # input tiles (kept resident; wk/x first so K/Q proj of pair 0 starts early)
        def load_w(wdram, label):
            wts = []
            for dt in range(8):
                w = work.tile([128, O], BF16, name=f"w{label}{dt}", tag=f"w{label}{dt}")
                nc.sync.dma_start(out=w, in_=wdram[dt * 128 : (dt + 1) * 128, :])
                wts.append(w)
            return wts

        # DMA order: wk, first i-slice of x (k c=0 can start), wq (q c=0),
        # wv (V projection feeds pair-0 chunk 0), then the rest of x
        wk_t = load_w(wkt, "k")
        xts = [work.tile([128, S], BF16, name=f"xts{dt}", tag=f"xts{dt}") for dt in range(8)]
        for dt in range(8):
            nc.sync.dma_start(out=xts[dt][:, 0:512], in_=xt[dt * 128 : (dt + 1) * 128, 0:512])
        wq_t = load_w(wqt, "q")
        wv_t = load_w(wvt, "v")
        for dt in range(8):
            nc.sync.dma_start(out=xts[dt][:, 512:S], in_=xt[dt * 128 : (dt + 1) * 128, 512:S])

        def kq_proj_steps(p, c0=0, kq_outer=False):
            """Generator: Q/K projection of pair p, one matmul per yield.
            kq_outer emits all K chunks before all Q chunks (K columns are
            read by every j-tile of the pair's first chunk, Q per i-chunk)."""
            kq = ((wk_t, kts, bk_sb), (wq_t, qts, bq_sb))
            cs = range(c0, 4)
            order = (
                [(w, c) for w in kq for c in cs]
                if kq_outer
                else [(w, c) for c in cs for w in kq]
            )
            for (wts, dsts, bias_sb), c in order:
                ps = psum.tile([128, 512], F32, name=f"pp{p}_{c}", tag="pp", bufs=2)
                for dt in range(8):
                    nc.tensor.matmul(
                        ps,
                        wts[dt][:, p * 128 : (p + 1) * 128],
                        xts[dt][:, c * 512 : (c + 1) * 512],
                        start=(dt == 0),
                        stop=(dt == 7),
                    )
                    if dt == 7:
                        nc.vector.tensor_scalar_add(
                            dsts[p][:, c * 512 : (c + 1) * 512],
                            ps,
                            bias_sb[:, p : p + 1],
                        )
                    yield

        def vproj_steps():
            """Generator: V projection + V_aug assembly, one s-tile group per
            yield. vaug[st][:, h, 0] = 1, [:, h, 64:128] = V + bv."""
            for st in range(ST):
                ps = psum.tile([128, O], F32, name=f"ppv{st}", tag="pp", bufs=2)
                for dt in range(8):
                    nc.tensor.matmul(
                        ps,
                        xts[dt][:, st * 128 : (st + 1) * 128],
                        wv_t[dt],
                        start=(dt == 0),
                        stop=(dt == 7),
                    )
                va = vaug[st]
                nc.vector.memset(va[:, :, 0:1], 1.0)
                nc.vector.memset(va[:, :, 1:DH], 0.0)
                nc.vector.tensor_add(
                    va[:, :, DH : 2 * DH],
                    ps.rearrange("p (h d) -> p h d", h=HL),
                    bv_bc.rearrange("p (h d) -> p h d", h=HL),
                )
                yield True

        # pair-0 Q/K projection up front; later pairs' projections are
        # interleaved one matmul per attention slot
        for _ in kq_proj_steps(0):
            pass
        vgen = vproj_steps()

        # attention per pair, with pair p+1's Q/K projection interleaved
        for p in range(NP):
            gen = kq_proj_steps(p + 1) if p + 1 < NP else None
            for ic in range(NIC):
                isl = slice(ic * 512, (ic + 1) * 512)
                cxs = [
                    psum.tile([128, 512], F32, name=f"cx{p}_{ic}_{x}", tag="cx", bufs=2)
                    for x in range(2)
                ]
                def emit_ctx(jt, u):
                    for x in range(2):
                        nc.tensor.matmul(
                            cxs[x],
                            vaug[jt][:, 2 * p + x, :],
                            u[:, x, :],
                            start=(jt == 0),
                            stop=(jt == ST - 1),
                        )

                prev = None
                for jt in range(ST):
                    s = psum.tile(
                        [128, 2, 512], F32, name=f"s{p}_{ic}_{jt}", tag="s", bufs=2
                    )
                    for x in range(2):
                        hp = slice(x * 64, (x + 1) * 64)
                        nc.tensor.matmul(
                            s[:, x, :],
                            kts[p][hp, jt * 128 : (jt + 1) * 128],
                            qts[p][hp, isl],
                            start=True,
                            stop=True,
                            tile_position=(x * 64, 0),
                        )
                    u = work.tile([128, 2, 512], BF16, name=f"u{p}_{ic}_{jt}", tag="u", bufs=12)
                    nc.scalar.activation(
                        u.rearrange("p x i -> p (x i)"),
                        s.rearrange("p x i -> p (x i)"),
                        EXP,
                        bias=mask_sb[:, jt : jt + 1],
                        scale=0.125,
                    )
                    # V projection rides in pair 0's first chunk, one s-tile
                    # group per slot, just ahead of the ctx matmul that needs it
                    next(vgen, None)
                    # always-ready projection work fills PE time while exp runs
                    if gen is not None:
                        next(gen, None)
                    # ctx one slot behind: its exp has already finished, so the
                    # PE queue never blocks a full exp latency mid-slot
                    if prev is not None:
                        emit_ctx(*prev)
                    prev = (jt, u)
                emit_ctx(*prev)
                # drain: row 0 of cxs = softmax denominator, rows 64:128 = ctx.
                # Copy PSUM->SBUF immediately (releases the bank in ~0.4us so
                # the next chunk's ctx accumulation isn't stalled), then
                # normalize off the critical path.
                for x in range(2):
                    st_ = work.tile([128, 512], F32, name=f"st{p}_{ic}_{x}", tag="st", bufs=3)
                    nc.vector.tensor_copy(out=st_, in_=cxs[x])
                    rd = work.tile([1, 512], F32, name=f"rd{p}_{ic}_{x}", tag="rd", bufs=2)
                    nc.vector.reciprocal_approx_fast(out=rd, in_=st_[0:1, :])
                    rdb = work.tile([128, 512], F32, name=f"rdb{p}_{ic}_{x}", tag="rdb", bufs=2)
                    nc.gpsimd.partition_broadcast(rdb, rd, channels=128)
                    ob = work.tile([128, 512], BF16, name=f"ob{p}_{ic}_{x}", tag="ob", bufs=3)
                    nc.vector.tensor_mul(
                        ob[DH:128, :], st_[DH:128, :], rdb[DH:128, :]
                    )
                    hh = 2 * p + x
                    nc.sync.dma_start(
                        out=out[hh * DH : (hh + 1) * DH, isl], in_=ob[DH:128, :]
                    )


def _make_in_maps(hidden_states, attention_mask, Wq, bq, Wk, bk, Wv, bv):
    bf = ml_dtypes.bfloat16
    in_maps = []
    for c in range(8):
        b, hg = divmod(c, 2)
        sl = slice(hg * O, (hg + 1) * O)
        in_maps.append(
            {
                "xt": np.ascontiguousarray(hidden_states[b].T).astype(bf),
                "wqt": np.ascontiguousarray(Wq[sl, :].T).astype(bf),
                "wkt": np.ascontiguousarray(Wk[sl, :].T).astype(bf),
                "wvt": np.ascontiguousarray(Wv[sl, :].T).astype(bf),
                "bq": np.ascontiguousarray(bq[sl]),
                "bk": np.ascontiguousarray(bk[sl]),
                "bv": np.ascontiguousarray(bv[sl]),
                "mask": np.ascontiguousarray(attention_mask[b, 0, 0, :]),
            }
        )
    return in_maps


def _gather(results):
    out = np.empty((B, S, D), dtype=np.float32)
    for c in range(8):
        b, hg = divmod(c, 2)
        out[b, :, hg * O : (hg + 1) * O] = results[c]["out"].astype(np.float32).T
    return out


def kernel(hidden_states, attention_mask, Wq, bq, Wk, bk, Wv, bv, **run_kwargs):
    global _NC_CACHE
    args = [hidden_states, attention_mask, Wq, bq, Wk, bk, Wv, bv]
    args = [np.asarray(a, dtype=np.float32) for a in args]
    if _NC_CACHE is None:
        _NC_CACHE = build_nc()
    in_maps = _make_in_maps(*args)
    res = run_bass_kernel_spmd(_NC_CACHE, in_maps, core_ids=list(range(8)), **run_kwargs)
    kernel.last_result = res
    return _gather(res.results)


# revision 22
# speedup vs baseline: 1.0317x; 1.0095x over previous
"""BERT self-attention (B=4, S=2048, D=1024, H=16) on 8 trn2 NeuronCores.

Sharding: core c -> (batch b = c//2, head-group hg = c%2, 8 heads each).
Each core computes out[b, :, hg*512:(hg+1)*512]; host gathers. Inputs are
pre-transposed AND cast to bf16 on host so matmuls run at full PE rate
(fp32 feed is half rate): xt = X.T [D,S] bf16, w{q,k,v}t = W.T [D,512] bf16.

On-device per core, all matmul operands bf16, PSUM accumulation fp32:
  Q^T, K^T: [o, s] pair-tiles (2 heads / 128 partitions); V_aug [s, h, 65]
  with a leading ones column per head (row 0 of ctx psum = softmax denom).
  Scores S^T[j, i] per head pair packed into one [128, 2, 512] PSUM tile
  (the two heads' K=64 matmuls run concurrently in disjoint PE row groups).
  U = exp(0.125*S^T + mask[j]) in ONE activation instr over 1024 free
  elems -> bf16 SBUF.
  ctx_aug[1+dh, i] accumulated over all 16 j-tiles directly in PSUM.
  Drain: DVE reciprocal of denom row, GpSimd partition-broadcast, DVE
  multiply -> bf16 out tile. Output is stored TRANSPOSED [o, s] in DRAM;
  the host transposes for free during the gather.
  Q/K projections for pair p+1 are interleaved into pair p's attention
  loop (1 matmul per score-tile slot) to fill the ACT-bound PE slack.
"""

import numpy as np
import ml_dtypes

import concourse.bass as bass
import concourse.tile as tile
from concourse import bacc, mybir
from concourse.bass_utils import run_bass_kernel_spmd

B, S, D, H = 4, 2048, 1024, 16
DH = 64
O = 512  # per-core output width (8 heads)
HL = 8  # local heads per core
NP = 4  # head pairs per core
ST = S // 128  # 16 s-tiles
NIC = 4  # i-chunks of 512
F32 = mybir.dt.float32
BF16 = mybir.dt.bfloat16
EXP = mybir.ActivationFunctionType.Exp

_NC_CACHE = None


def build_nc():
    nc = bacc.Bacc(
        "TRN2",
        target_bir_lowering=False,
        debug=False,
        enable_asserts=True,
        num_devices=8,
    )
    xt = nc.dram_tensor("xt", [D, S], BF16, kind="ExternalInput").ap()
    wqt = nc.dram_tensor("wqt", [D, O], BF16, kind="ExternalInput").ap()
    wkt = nc.dram_tensor("wkt", [D, O], BF16, kind="ExternalInput").ap()
    wvt = nc.dram_tensor("wvt", [D, O], BF16, kind="ExternalInput").ap()
    bq = nc.dram_tensor("bq", [O], F32, kind="ExternalInput").ap()
    bk = nc.dram_tensor("bk", [O], F32, kind="ExternalInput").ap()
    bv = nc.dram_tensor("bv", [O], F32, kind="ExternalInput").ap()
    mask = nc.dram_tensor("mask", [S], F32, kind="ExternalInput").ap()
    # transposed output: [o, s]; host transposes during gather
    out = nc.dram_tensor("out", [O, S], BF16, kind="ExternalOutput").ap()

    with tile.TileContext(nc) as tc:
        _emit(nc, tc, xt, wqt, wkt, wvt, bq, bk, bv, mask, out)
    nc.compile()
    return nc


def _emit(nc, tc, xt, wqt, wkt, wvt, bq, bk, bv, mask, out):
    with (
        tc.tile_pool(name="singles", bufs=1) as singles,
        tc.tile_pool(name="persist", bufs=1) as persist,
        tc.tile_pool(name="work", bufs=1) as work,
        tc.tile_pool(name="psum", bufs=1, space="PSUM") as psum,
    ):
        mask_sb = singles.tile([128, ST], F32)
        nc.sync.dma_start(out=mask_sb, in_=mask.rearrange("(t p) -> p t", p=128))
        bq_sb = singles.tile([128, NP], F32)
        nc.sync.dma_start(out=bq_sb, in_=bq.rearrange("(t p) -> p t", p=128))
        bk_sb = singles.tile([128, NP], F32)
        nc.sync.dma_start(out=bk_sb, in_=bk.rearrange("(t p) -> p t", p=128))
        bv_bc = singles.tile([128, O], F32)
        nc.sync.dma_start(
            out=bv_bc, in_=bass.AP(tensor=bv.tensor, offset=0, ap=[[0, 128], [1, O]])
        )
        # warm the exp table set while the input DMAs stream
        warm_in = singles.tile([128, 1], F32)
        nc.vector.memset(warm_in, 0.0)
        warm_out = singles.tile([128, 1], F32)
        nc.scalar.activation(warm_out, warm_in, EXP)

        # persistent activations. vaug layout per head: col 0 = ones (softmax
        # denominator via the ctx matmul), cols 1:64 = zero pad (PE partition
        # slices must start 32-aligned, so V sits at rows 64:128 of ctx psum),
        # cols 64:128 = V + bias.
        qts = [persist.tile([128, S], BF16, name=f"qt{p}", tag=f"qt{p}") for p in range(NP)]
        kts = [persist.tile([128, S], BF16, name=f"kt{p}", tag=f"kt{p}") for p in range(NP)]
        vaug = [
            persist.tile([128, HL, 128], BF16, name=f"vaug{t}", tag=f"vaug{t}")
            for t in range(ST)
        ]

        # input tiles (kept resident; wk/x first so K/Q proj of pair 0 starts early)
        def load_w(wdram, label):
            wts = []
            for dt in range(8):
                w = work.tile([128, O], BF16, name=f"w{label}{dt}", tag=f"w{label}{dt}")
                nc.sync.dma_start(out=w, in_=wdram[dt * 128 : (dt + 1) * 128, :])
                wts.append(w)
            return wts

        # DMA order: wk, first i-slice of x (k c=0 can start), wq (q c=0),
        # wv (V projection feeds pair-0 chunk 0), then the rest of x
        wk_t = load_w(wkt, "k")
        xts = [work.tile([128, S], BF16, name=f"xts{dt}", tag=f"xts{dt}") for dt in range(8)]
        for dt in range(8):
            nc.sync.dma_start(out=xts[dt][:, 0:512], in_=xt[dt * 128 : (dt + 1) * 128, 0:512])
        wq_t = load_w(wqt, "q")
        for dt in range(8):
            nc.sync.dma_start(out=xts[dt][:, 512:S], in_=xt[dt * 128 : (dt + 1) * 128, 512:S])
        wv_t = load_w(wvt, "v")

        def kq_proj_steps(p, c0=0, kq_outer=False):
            """Generator: Q/K projection of pair p, one matmul per yield.
            kq_outer emits all K chunks before all Q chunks (K columns are
            read by every j-tile of the pair's first chunk, Q per i-chunk)."""
            kq = ((wk_t, kts, bk_sb), (wq_t, qts, bq_sb))
            cs = range(c0, 4)
            order = (
                [(w, c) for w in kq for c in cs]
                if kq_outer
                else [(w, c) for c in cs for w in kq]
            )
            for (wts, dsts, bias_sb), c in order:
                ps = psum.tile([128, 512], F32, name=f"pp{p}_{c}", tag="pp", bufs=2)
                for dt in range(8):
                    nc.tensor.matmul(
                        ps,
                        wts[dt][:, p * 128 : (p + 1) * 128],
                        xts[dt][:, c * 512 : (c + 1) * 512],
                        start=(dt == 0),
                        stop=(dt == 7),
                    )
                    if dt == 7:
                        nc.vector.tensor_scalar_add(
                            dsts[p][:, c * 512 : (c + 1) * 512],
                            ps,
                            bias_sb[:, p : p + 1],
                        )
                    yield

        def vproj_steps():
            """Generator: V projection + V_aug assembly, one s-tile group per
            yield. vaug[st][:, h, 0] = 1, [:, h, 64:128] = V + bv."""
            for st in range(ST):
                ps = psum.tile([128, O], F32, name=f"ppv{st}", tag="pp", bufs=2)
                for dt in range(8):
                    nc.tensor.matmul(
                        ps,
                        xts[dt][:, st * 128 : (st + 1) * 128],
                        wv_t[dt],
                        start=(dt == 0),
                        stop=(dt == 7),
                    )
                va = vaug[st]
                nc.vector.memset(va[:, :, 0:1], 1.0)
                nc.vector.memset(va[:, :, 1:DH], 0.0)
                nc.vector.tensor_add(
                    va[:, :, DH : 2 * DH],
                    ps.rearrange("p (h d) -> p h d", h=HL),
                    bv_bc.rearrange("p (h d) -> p h d", h=HL),
                )
                yield True

        # pair-0 Q/K projection up front; later pairs' projections are
        # interleaved one matmul per attention slot
        for _ in kq_proj_steps(0):
            pass
        vgen = vproj_steps()

        # attention per pair, with pair p+1's Q/K projection interleaved
        for p in range(NP):
            gen = kq_proj_steps(p + 1) if p + 1 < NP else None
            for ic in range(NIC):
                isl = slice(ic * 512, (ic + 1) * 512)
                cxs = [
                    psum.tile([128, 512], F32, name=f"cx{p}_{ic}_{x}", tag="cx", bufs=2)
                    for x in range(2)
                ]
                def emit_ctx(jt, u):
                    for x in range(2):
                        nc.tensor.matmul(
                            cxs[x],
                            vaug[jt][:, 2 * p + x, :],
                            u[:, x, :],
                            start=(jt == 0),
                            stop=(jt == ST - 1),
                        )

                prev = None
                for jt in range(ST):
                    s = psum.tile(
                        [128, 2, 512], F32, name=f"s{p}_{ic}_{jt}", tag="s", bufs=2
                    )
                    for x in range(2):
                        hp = slice(x * 64, (x + 1) * 64)
                        nc.tensor.matmul(
                            s[:, x, :],
                            kts[p][hp, jt * 128 : (jt + 1) * 128],
                            qts[p][hp, isl],
                            start=True,
                            stop=True,
                            tile_position=(x * 64, 0),
                        )
                    u = work.tile([128, 2, 512], BF16, name=f"u{p}_{ic}_{jt}", tag="u", bufs=16)
                    nc.scalar.activation(
                        u.rearrange("p x i -> p (x i)"),
                        s.rearrange("p x i -> p (x i)"),
                        EXP,
                        bias=mask_sb[:, jt : jt + 1],
                        scale=0.125,
                    )
                    # V projection rides in pair 0's first chunk, one s-tile
                    # group per slot, just ahead of the ctx matmul that needs it
                    next(vgen, None)
                    # always-ready projection work fills PE time while exp runs
                    if gen is not None:
                        next(gen, None)
                    # ctx one slot behind: its exp has already finished, so the
                    # PE queue never blocks a full exp latency mid-slot
                    if prev is not None:
                        emit_ctx(*prev)
                    prev = (jt, u)
                emit_ctx(*prev)
                # drain: row 0 of cxs = softmax denominator, rows 64:128 = ctx.
                # Copy PSUM->SBUF immediately (releases the bank in ~0.4us so
                # the next chunk's ctx accumulation isn't stalled), then
                # normalize off the critical path.
                for x in range(2):
                    st_ = work.tile([128, 512], F32, name=f"st{p}_{ic}_{x}", tag="st", bufs=4)
                    nc.vector.tensor_copy(out=st_, in_=cxs[x])
                    rd = work.tile([1, 512], F32, name=f"rd{p}_{ic}_{x}", tag="rd", bufs=2)
                    nc.vector.reciprocal_approx_fast(out=rd, in_=st_[0:1, :])
                    rdb = work.tile([128, 512], F32, name=f"rdb{p}_{ic}_{x}", tag="rdb", bufs=2)
                    nc.gpsimd.partition_broadcast(rdb, rd, channels=128)
                    ob = work.tile([128, 512], BF16, name=f"ob{p}_{ic}_{x}", tag="ob", bufs=4)
                    nc.vector.tensor_mul(
                        ob[DH:128, :], st_[DH:128, :], rdb[DH:128, :]
                    )
                    hh = 2 * p + x
                    nc.sync.dma_start(
                        out=out[hh * DH : (hh + 1) * DH, isl], in_=ob[DH:128, :]
                    )


def _make_in_maps(hidden_states, attention_mask, Wq, bq, Wk, bk, Wv, bv):
    bf = ml_dtypes.bfloat16
    in_maps = []
    for c in range(8):
        b, hg = divmod(c, 2)
        sl = slice(hg * O, (hg + 1) * O)
        in_maps.append(
            {
                "xt": np.ascontiguousarray(hidden_states[b].T).astype(bf),
                "wqt": np.ascontiguousarray(Wq[sl, :].T).astype(bf),
                "wkt": np.ascontiguousarray(Wk[sl, :].T).astype(bf),
                "wvt": np.ascontiguousarray(Wv[sl, :].T).astype(bf),
                "bq": np.ascontiguousarray(bq[sl]),
                "bk": np.ascontiguousarray(bk[sl]),
                "bv": np.ascontiguousarray(bv[sl]),
                "mask": np.ascontiguousarray(attention_mask[b, 0, 0, :]),
            }
        )
    return in_maps


def _gather(results):
    out = np.empty((B, S, D), dtype=np.float32)
    for c in range(8):
        b, hg = divmod(c, 2)
        out[b, :, hg * O : (hg + 1) * O] = results[c]["out"].astype(np.float32).T
    return out


def kernel(hidden_states, attention_mask, Wq, bq, Wk, bk, Wv, bv, **run_kwargs):
    global _NC_CACHE
    args = [hidden_states, attention_mask, Wq, bq, Wk, bk, Wv, bv]
    args = [np.asarray(a, dtype=np.float32) for a in args]
    if _NC_CACHE is None:
        _NC_CACHE = build_nc()
    in_maps = _make_in_maps(*args)
    res = run_bass_kernel_spmd(_NC_CACHE, in_maps, core_ids=list(range(8)), **run_kwargs)
    kernel.last_result = res
    return _gather(res.results)
